# revision 1
# baseline (speedup 1.0000x reference)
"""Conditional_Embedding_Contrastive_loss Trainium2 kernel.

Full-input contract: kernel(**inputs) takes the complete tensors, shards
rows across 8 NeuronCores (data-parallel), runs one SPMD Bass/Tile kernel,
and reduces the per-row log-ratios to the scalar loss on the host.

Math (reference, augmentation=None branch):
    sim   = cosine_sim(X, X)                      # [N,N]
    IZ    = exp(offdiag(sim)/T)                   # [N,N-1]
    Mneg  = offdiag(cls_mask[labels])             # [N,N-1]
    p     = exp(cos(x_i, a_i)/T)                  # [N]
    num_i = sum_j IZ*Mneg + p_i
    den_i = p_i + sum_j IZ
    loss  = -mean(log(num_i/den_i))

Since cos(x,x) == 1 exactly, the diagonal removal is analytic:
    sum_offdiag exp(sim/T)        = S_all_i - exp(1/T)
    sum_offdiag exp(sim/T)*m      = S_msk_i - exp(1/T)*m_ii
so each core computes full row sums of its [R, N] slice of exp(sim/T)
(and the masked variant) plus p_i, then:
    logq_i = ln(S_msk_i - E0*m_ii + p_i) - ln(S_all_i - E0 + p_i)
Host: loss = -mean(logq).

Device pipeline per core (R = N/8 = 512 rows):
  - G tile [128,512] = lhsT.T @ rhs over 8 k-chunks (bf16 PE matmul);
    lhsT = own-column slice of X^T, rhs = full X^T (both SBUF resident).
  - norms: squares (DVE) + ones-matmul partition-reduce (PE) -> n2 [1,N];
    r = exp(-0.5*ln(n2)) on ACT (stays in the ln/exp table set);
    r broadcast to [128,N] via stride-0 DMA through a DRAM scratch.
  - per tile: h = (G * r_i) * r_j (one DVE scalar_tensor_tensor),
    e = exp(h/T) on ACT with accum_out = unmasked row-sum (free),
    masked row-sum via one DVE scalar_tensor_tensor (accum_out)
    against the host-gathered mask slice cls_mask[labels[rows]].
  - p_i from row-major own slices: dots/norms via DVE reduce, exp on ACT.
"""

import sys

for _p in ("/opt/trn_rl_repo",):
    if _p not in sys.path:
        sys.path.insert(0, _p)

import numpy as np
import ml_dtypes

P = 128  # SBUF partitions
JW = 512  # j-tile width (one PSUM bank of fp32)

_CACHE = {}

def build_kernel(N, D, R, inv_T, mm_bf16=True, n_cores=8,
                 mpsum_bufs=3, work_bufs=3, mask_bufs=3, sq_bufs=2,
                 mask_engine="vector", xt_split=2, mask_dma="sync",
                 late_xsanc=False, skip_norm=False, skip_mm=False,
                 e_bf16=False, npsum_bufs=1, post_jt=2048):
    """Build the SPMD Bass program for one core owning R rows of N total."""
    import concourse.bass as bass
    import concourse.mybir as mybir
    import concourse.tile as tile
    from concourse import bacc

    f32 = mybir.dt.float32
    bf16 = mybir.dt.bfloat16
    MMDT = bf16 if mm_bf16 else f32
    Exp = mybir.ActivationFunctionType.Exp
    Ln = mybir.ActivationFunctionType.Ln
    Sq = mybir.ActivationFunctionType.Square
    mult = mybir.AluOpType.mult
    add = mybir.AluOpType.add
    X = mybir.AxisListType.X

    E0 = float(np.exp(inv_T))  # exp(1/T): the analytic diagonal term

    KC = D // P   # contraction chunks of 128
    NB = R // P   # own row blocks
    JT = min(1024, N)  # main tile width (2 PSUM banks)
    JC = N // JT  # main j tiles per row block
    NH = JT // JW  # matmul groups per tile (N=512 each)

    nc = bacc.Bacc(
        "TRN2", target_bir_lowering=False, debug=False, num_devices=n_cores)
    xt_d = nc.declare_dram_parameter("xt", [D, N], MMDT, isOutput=False)
    xst_d = nc.declare_dram_parameter("xst", [D, R], MMDT, isOutput=False)
    xs_d = nc.declare_dram_parameter("xs", [R, D], f32, isOutput=False)
    anc_d = nc.declare_dram_parameter("anc", [R, D], f32, isOutput=False)
    mt_d = nc.declare_dram_parameter("mt", [R, N], bf16, isOutput=False)
    md_d = nc.declare_dram_parameter("mdiag", [NB, P, 1], f32, isOutput=False)
    out_d = nc.declare_dram_parameter("logq", [NB, P, 1], f32, isOutput=True)

    mask_eng = nc.gpsimd if mask_engine == "gpsimd" else nc.vector

    with tile.TileContext(nc) as tc:
        with (
            tc.tile_pool(name="big", bufs=1) as big,
            tc.tile_pool(name="sq", bufs=sq_bufs) as sqp,
            tc.tile_pool(name="mask", bufs=mask_bufs) as maskp,
            tc.tile_pool(name="work", bufs=work_bufs) as workp,
            tc.tile_pool(name="stats", bufs=1) as statsp,
            tc.tile_pool(name="tiny", bufs=2) as tinyp,
            tc.tile_pool(name="rdr", bufs=1, space="DRAM") as dramp,
            tc.tile_pool(name="npsum", bufs=npsum_bufs, space="PSUM") as npsum,
            tc.tile_pool(name="mpsum", bufs=mpsum_bufs, space="PSUM") as mpsum,
        ):
            xt_sb = big.tile([P, KC, N], MMDT)
            xst_sb = big.tile([P, KC, R], MMDT)
            xs_sb = big.tile([P, NB, D], f32)
            anc_sb = big.tile([P, NB, D], f32)
            rbc = big.tile([P, N], f32)
            md_sb = statsp.tile([P, NB], f32)
            ones_w = statsp.tile([P, 1], MMDT)
            JGw = max(1, JC // max(1, min(post_jt, N) // JT))
            accA = statsp.tile([P, NB, JGw], f32)
            accM = statsp.tile([P, NB, JGw], f32)
            rq = statsp.tile([P, NB], f32)    # r_i = 1/||x_i||
            pvec = statsp.tile([P, NB], f32)  # p_i
            logq = statsp.tile([P, NB], f32)
            rdram = dramp.tile([1, N], f32)

            # ---- input DMAs, all on the HW DGE queue ----
            for c in range(KC):
                for s in range(xt_split):
                    w = N // xt_split
                    nc.sync.dma_start(
                        xt_sb[:, c, s * w : (s + 1) * w],
                        xt_d[c * P : (c + 1) * P, s * w : (s + 1) * w])
                nc.sync.dma_start(xst_sb[:, c, :], xst_d[c * P : (c + 1) * P, :])

            def load_xs_anc():
                for b in range(NB):
                    nc.sync.dma_start(xs_sb[:, b, :], xs_d[b * P : (b + 1) * P, :])
                    nc.sync.dma_start(
                        anc_sb[:, b, :], anc_d[b * P : (b + 1) * P, :])
                    nc.sync.dma_start(md_sb[:, b : b + 1], md_d[b])

            if not late_xsanc:
                load_xs_anc()
            nc.vector.memset(ones_w[:], 1.0)

            # Pre-place the combined ln+exp activation table so the compiler
            # doesn't flip-flop between the exp-only and ln-only sets
            # (each switch costs ~2.7us on the scalar engine).
            ACT_SET_LN_EXP = 6  # natural_log_exp_and_others (gen3 act_info)
            nc.scalar.add_instruction(mybir.InstLoadActFuncSet(
                name=nc.get_next_instruction_name(),
                act_func_set_id=ACT_SET_LN_EXP, ins=[], outs=[]))

            # ---- norms of all N columns -> r broadcast tile ----
            # n2_j = sum_d x_dj^2 via DVE squares + ones-matmul partition
            # reduce; r_j = exp(-0.5*ln(n2_j)) (stays in one ACT table set);
            # broadcast through DRAM with a stride-0 partition read.
            if skip_norm:
                nc.vector.memset(rbc[:], 0.03)
            for jq in range(JC if not skip_norm else 0):
                n2q = npsum.tile([1, JT], f32, tag="n2q", name="n2q")
                for c in range(KC):
                    sqt = sqp.tile([P, JT], MMDT, tag="sqt", name="sqt")
                    xsl = xt_sb[:, c, jq * JT : (jq + 1) * JT]
                    nc.vector.tensor_mul(sqt, xsl, xsl)
                    for h in range(NH):
                        nc.tensor.matmul(
                            n2q[:, h * JW : (h + 1) * JW], ones_w[:],
                            sqt[:, h * JW : (h + 1) * JW],
                            start=(c == 0), stop=(c == KC - 1))
                lnr = tinyp.tile([1, JT], f32, tag="lnr")
                nc.scalar.activation(lnr, n2q[:], Ln)
                rr = tinyp.tile([1, JT], f32, tag="rr")
                nc.scalar.activation(rr, lnr, Exp, scale=-0.5)
                nc.sync.dma_start(rdram[0:1, jq * JT : (jq + 1) * JT], rr)
                rsl = rdram[0:1, jq * JT : (jq + 1) * JT]
                bc = bass.AP(tensor=rsl.tensor, offset=rsl.offset,
                             ap=[[0, P], [1, JT]])
                nc.sync.dma_start(rbc[:, jq * JT : (jq + 1) * JT], bc)

            # ---- p path: p_i = exp(dot_i/(n_i*na_i*T)); also r_i ----
            if late_xsanc:
                load_xs_anc()
            for b in range(NB):
                xb = xs_sb[:, b, :]
                ab = anc_sb[:, b, :]
                n2x = tinyp.tile([P, 1], f32, tag="n2x")
                n2a = tinyp.tile([P, 1], f32, tag="n2a")
                dotv = tinyp.tile([P, 1], f32, tag="dotv")
                j1 = workp.tile([P, D], f32, tag="pjunk")
                nc.scalar.activation(j1, xb, Sq, accum_out=n2x)
                j2 = workp.tile([P, D], f32, tag="pjunk")
                nc.scalar.activation(j2, ab, Sq, accum_out=n2a)
                j3 = workp.tile([P, D], f32, tag="pjunk")
                nc.vector.scalar_tensor_tensor(
                    out=j3, in0=xb, scalar=1.0, in1=ab, op0=mult, op1=mult,
                    accum_out=dotv)
                l1 = tinyp.tile([P, 1], f32, tag="l1")
                l2 = tinyp.tile([P, 1], f32, tag="l2")
                nc.scalar.activation(l1, n2x, Ln)
                nc.scalar.activation(l2, n2a, Ln)
                # r_i = exp(-0.5*ln(n2x))
                nc.scalar.activation(rq[:, b : b + 1], l1, Exp, scale=-0.5)
                ls = tinyp.tile([P, 1], f32, tag="ls")
                nc.vector.tensor_add(ls, l1, l2)
                qv = tinyp.tile([P, 1], f32, tag="qv")
                nc.scalar.activation(qv, ls, Exp, scale=-0.5)  # 1/(n_i*na_i)
                q2 = tinyp.tile([P, 1], f32, tag="q2")
                nc.vector.tensor_scalar_mul(q2, qv, float(inv_T))
                nc.scalar.activation(pvec[:, b : b + 1], dotv, Exp, scale=q2)

            # ---- main: G tiles -> exp -> masked/unmasked row sums ----
            PJ = min(post_jt, N)   # post-processing group width
            PG = max(1, PJ // JT)  # psum tiles per group
            JG = JC // PG          # groups per row block
            for b in range(NB):
                for g in range(JG):
                    h2 = workp.tile([P, PJ], f32, tag="h2", bufs=2, name="h2")
                    for q in range(PG):
                        jc = g * PG + q
                        ps = mpsum.tile([P, JT], f32, tag="ps", name="ps")
                        if skip_mm:
                            nc.vector.memset(ps[:], 0.5)
                        for c in range(KC if not skip_mm else 0):
                            for h in range(NH):
                                nc.tensor.matmul(
                                    ps[:, h * JW : (h + 1) * JW],
                                    xst_sb[:, c, b * P : (b + 1) * P],
                                    xt_sb[:, c,
                                          jc * JT + h * JW : jc * JT + (h + 1) * JW],
                                    start=(c == 0), stop=(c == KC - 1))
                        nc.vector.scalar_tensor_tensor(
                            out=h2[:, q * JT : (q + 1) * JT], in0=ps[:],
                            scalar=rq[:, b : b + 1],
                            in1=rbc[:, jc * JT : (jc + 1) * JT],
                            op0=mult, op1=mult)
                    mtt = maskp.tile([P, PJ], bf16, tag="mtt", bufs=2, name="mtt")
                    (nc.gpsimd if mask_dma == "gpsimd" else nc.sync).dma_start(
                        mtt, mt_d[b * P : (b + 1) * P, g * PJ : (g + 1) * PJ])
                    e = workp.tile([P, PJ], bf16 if e_bf16 else f32, tag="e",
                                   bufs=2, name="e")
                    nc.scalar.activation(
                        e, h2, Exp, scale=float(inv_T),
                        accum_out=accA[:, b, g : g + 1])
                    # junk elementwise product written over h2 (dead after exp)
                    mask_eng.scalar_tensor_tensor(
                        out=h2, in0=e, scalar=1.0, in1=mtt, op0=mult, op1=mult,
                        accum_out=accM[:, b, g : g + 1])
                # tail: assemble logq for block b
                sA = tinyp.tile([P, 1], f32, tag="sA")
                sM = tinyp.tile([P, 1], f32, tag="sM")
                nc.vector.reduce_sum(sA, accA[:, b, :], axis=X)
                nc.vector.reduce_sum(sM, accM[:, b, :], axis=X)
                num = tinyp.tile([P, 1], f32, tag="num")
                # num = sM - E0*mdiag  (then + p)
                nc.vector.scalar_tensor_tensor(
                    out=num, in0=md_sb[:, b : b + 1], scalar=-E0, in1=sM,
                    op0=mult, op1=add)
                num2 = tinyp.tile([P, 1], f32, tag="num2")
                nc.vector.tensor_add(num2, num, pvec[:, b : b + 1])
                den = tinyp.tile([P, 1], f32, tag="den")
                nc.vector.tensor_add(den, sA, pvec[:, b : b + 1])
                den2 = tinyp.tile([P, 1], f32, tag="den2")
                nc.vector.tensor_scalar_add(den2, den, -E0)
                lnn = tinyp.tile([P, 1], f32, tag="lnn")
                lnd = tinyp.tile([P, 1], f32, tag="lnd")
                nc.scalar.activation(lnn, num2, Ln)
                nc.scalar.activation(lnd, den2, Ln)
                nc.vector.tensor_sub(logq[:, b : b + 1], lnn, lnd)
                nc.sync.dma_start(out_d[b], logq[:, b : b + 1])

    nc.compile()
    return nc


def _prepare_inputs(inst_embed, anchor, cls_mask, labels, n_cores):
    """Host-side sharding/marshalling: slices, transpose, mask gather, casts."""
    N, D = inst_embed.shape
    R = N // n_cores
    bf = ml_dtypes.bfloat16
    Xf = np.ascontiguousarray(inst_embed, dtype=np.float32)
    Af = np.ascontiguousarray(anchor, dtype=np.float32)
    XT = np.ascontiguousarray(Xf.T).astype(bf)
    lab = np.asarray(labels).astype(np.int64)
    in_maps = []
    for k in range(n_cores):
        r0 = k * R
        rows = slice(r0, r0 + R)
        mrows = cls_mask[lab[rows]]  # [R, N] int
        mdiag = mrows[np.arange(R), r0 + np.arange(R)].astype(np.float32)
        in_maps.append({
            "xt": XT,
            "xst": np.ascontiguousarray(XT[:, rows]),
            "xs": np.ascontiguousarray(Xf[rows]),
            "anc": np.ascontiguousarray(Af[rows]),
            "mt": np.ascontiguousarray(mrows.astype(bf)),
            "mdiag": np.ascontiguousarray(
                mdiag.reshape(R // P, P, 1)),
        })
    return in_maps


def run(inst_embed, anchor, cls_mask, labels, temperature,
        n_cores=8, trace=False, mm_bf16=True):
    """Build (cached), run on hardware, and reduce. Returns (loss, results)."""
    from concourse.bass_utils import run_bass_kernel_spmd

    N, D = inst_embed.shape
    R = N // n_cores
    inv_T = float(1.0 / np.float32(temperature))
    key = (N, D, R, inv_T, mm_bf16)
    if key not in _CACHE:
        _CACHE[key] = build_kernel(
            N, D, R, inv_T, mm_bf16=mm_bf16, n_cores=n_cores)
    nc = _CACHE[key]

    in_maps = _prepare_inputs(inst_embed, anchor, cls_mask, labels, n_cores)
    from concourse.bass_interp import get_hw_module
    hw_m = get_hw_module(nc.m)
    old_m = nc.m
    nc.m = hw_m
    try:
        res = run_bass_kernel_spmd(
            nc, in_maps, list(range(n_cores)), trace=trace)
    finally:
        nc.m = old_m
    vals = np.concatenate(
        [np.asarray(r["logq"], dtype=np.float32).reshape(-1) for r in res.results])
    loss = -np.mean(vals.astype(np.float64))
    return np.array(loss, dtype=np.float32), res


def kernel(inst_embed, anchor, cls_mask, labels, temperature):
    loss, _ = run(inst_embed, anchor, cls_mask, labels, temperature)
    return loss



# revision 4
# speedup vs baseline: 32.7253x; 32.7253x over previous
"""Conditional_Embedding_Contrastive_loss Trainium2 kernel (8 cores).

Full-input contract: kernel(**inputs) takes the complete tensors and
returns the scalar loss. End-to-end wall time is dominated by the axon
host->device tunnel (~40-90 MB/s) and per-call jit re-compilation in the
stock runner, so this implementation is built around minimizing both:

  1. Each core receives ONLY its own 1 MB shard of the row-normalized
     embedding matrix (bf16 X-hat^T, [D, R]); the full [D, N] operand is
     assembled on-device with a DRAM AllGather over NeuronLink.
  2. Row norms, the anchor cosine term p_i, and the analytic diagonal
     corrections are computed on the host (cheap O(N*D) numpy) and folded
     into two tiny per-row vectors:
         logq_i = ln(S_msk_i + cnum_i) - ln(S_all_i + cden_i)
     with cnum_i = p_i - exp(1/T)*m_ii, cden_i = p_i - exp(1/T), where
     S_all/S_msk are full-row sums of exp(sim/T) (resp. masked by
     cls_mask[labels_i]) including the diagonal.
  3. The 0/1 mask rows are bit-packed on the host (plane-major: byte k,
     bit b <-> column b*(N/8)+k) to 256 KB/core and unpacked on-device
     with one fused (>>b)&1 DVE op per plane.
  4. The shard_map jit is built once per process and cached, so warm
     calls skip tracing/lowering/compilation entirely.

Device pipeline per core (R = N/8 = 512 rows, P = 128):
  - DRAM AllGather: xst [D,R] -> xg [8*D, R] (logical [8][D][R]).
  - SBUF: xt_sb [128, D/128, N] bf16 (gathered, 8 MB), xst_sb own shard,
    packed mask, cnum/cden.
  - per row-block b (4) and j-tile (1024 cols): PE matmul (8 k-chunks,
    2x512-wide) -> PSUM; ACT exp(scale=1/T) PSUM->SBUF with accum_out =
    unmasked row-sum; DVE scalar_tensor_tensor e*mask with accum_out =
    masked row-sum against the unpacked mask tile.
  - tail per block: two Ln on ACT, subtract, DMA out logq [NB,P,1].
Host: loss = -mean(logq).
"""

import sys

for _p in ("/opt/trn_rl_repo",):
    if _p not in sys.path:
        sys.path.insert(0, _p)

import numpy as np
import ml_dtypes

P = 128          # SBUF partitions
JW = 512         # PE moving free-dim max
EPS = 1e-8

_CACHE = {}


def build_kernel(N, D, R, inv_T, n_cores=8, shared_cc_out=True,
                 mpsum_bufs=3, work_bufs=2, mask_bufs=2):
    """Build the SPMD Bass program for one core owning R rows of N total."""
    import concourse.bass as bass
    import concourse.mybir as mybir
    import concourse.tile as tile
    from concourse import bacc

    f32 = mybir.dt.float32
    bf16 = mybir.dt.bfloat16
    u8 = mybir.dt.uint8
    Exp = mybir.ActivationFunctionType.Exp
    Ln = mybir.ActivationFunctionType.Ln
    mult = mybir.AluOpType.mult
    shr = mybir.AluOpType.logical_shift_right
    band = mybir.AluOpType.bitwise_and
    X = mybir.AxisListType.X

    KC = D // P        # contraction chunks of 128
    NB = R // P        # own row blocks
    JT = min(1024, N)  # j-tile width (2 PSUM banks of fp32)
    JC = N // JT       # j tiles per row block
    NH = JT // JW      # matmuls per j-tile per k-chunk
    NPB = N // 8       # packed-mask bytes per row (one bit-plane's width)

    nc = bacc.Bacc(
        "TRN2", target_bir_lowering=False, debug=False, num_devices=n_cores)
    xst_d = nc.declare_dram_parameter("xst", [D, R], bf16, isOutput=False)
    mpk_d = nc.declare_dram_parameter("mpk", [R, NPB], u8, isOutput=False)
    cnum_d = nc.declare_dram_parameter("cnum", [NB, P, 1], f32, isOutput=False)
    cden_d = nc.declare_dram_parameter("cden", [NB, P, 1], f32, isOutput=False)
    out_d = nc.declare_dram_parameter("logq", [NB, P, 1], f32, isOutput=True)

    with tile.TileContext(nc) as tc:
        with (
            tc.tile_pool(name="big", bufs=1) as big,
            tc.tile_pool(name="mask", bufs=mask_bufs) as maskp,
            tc.tile_pool(name="work", bufs=work_bufs) as workp,
            tc.tile_pool(name="stats", bufs=1) as statsp,
            tc.tile_pool(name="tiny", bufs=2) as tinyp,
            tc.tile_pool(name="dram", bufs=1, space="DRAM") as dramp,
            tc.tile_pool(name="mpsum", bufs=mpsum_bufs, space="PSUM") as mpsum,
        ):
            xt_sb = big.tile([P, KC, N], bf16)
            xst_sb = big.tile([P, KC, R], bf16)
            mpk_sb = big.tile([P, NB, NPB], u8)
            cnum_sb = statsp.tile([P, NB], f32)
            cden_sb = statsp.tile([P, NB], f32)
            accA = statsp.tile([P, NB, JC], f32)
            accM = statsp.tile([P, NB, JC], f32)
            logq = statsp.tile([P, NB], f32)

            xin_b = dramp.tile([D, R], bf16)
            xg_b = dramp.tile(
                [n_cores * D, R], bf16,
                addr_space="Shared" if shared_cc_out else "Local")

            # ---- collective: own shard -> full gathered X-hat^T ----
            nc.sync.dma_start(xin_b[:], xst_d[:, :])
            nc.gpsimd.collective_compute(
                "AllGather", mybir.AluOpType.bypass,
                replica_groups=[list(range(n_cores))],
                ins=[xin_b.opt()], outs=[xg_b.opt()])

            # ---- input DMAs that don't depend on the collective ----
            for c in range(KC):
                nc.sync.dma_start(xst_sb[:, c, :], xst_d[c * P:(c + 1) * P, :])
            for b in range(NB):
                nc.sync.dma_start(mpk_sb[:, b, :], mpk_d[b * P:(b + 1) * P, :])
                nc.sync.dma_start(cnum_sb[:, b:b + 1], cnum_d[b])
                nc.sync.dma_start(cden_sb[:, b:b + 1], cden_d[b])

            # Pre-place the combined ln+exp activation table (a table switch
            # costs ~2.7us on the scalar engine).
            ACT_SET_LN_EXP = 6  # natural_log_exp_and_others (gen3 act_info)
            nc.scalar.add_instruction(mybir.InstLoadActFuncSet(
                name=nc.get_next_instruction_name(),
                act_func_set_id=ACT_SET_LN_EXP, ins=[], outs=[]))

            # ---- gathered shard -> SBUF ----
            for k in range(n_cores):
                for c in range(KC):
                    nc.sync.dma_start(
                        xt_sb[:, c, k * R:(k + 1) * R],
                        xg_b[k * D + c * P: k * D + (c + 1) * P, :])

            # ---- main loop ----
            for b in range(NB):
                # unpack this block's mask rows: bit-plane pl covers columns
                # [pl*NPB, (pl+1)*NPB). bitVec TSP ops can't cast dtypes, so
                # (>>pl)&1 stays u8->u8 and a mult-by-1 TSP does u8->bf16.
                m_sb = maskp.tile([P, N], bf16, tag="m", name="m_sb")
                for pl in range(8):
                    msh = maskp.tile([P, NPB], u8, tag="msh", name="msh")
                    nc.vector.tensor_scalar(
                        msh, mpk_sb[:, b, :], pl, 1, op0=shr, op1=band)
                    nc.vector.tensor_scalar_mul(
                        m_sb[:, pl * NPB:(pl + 1) * NPB], msh, 1)
                for jq in range(JC):
                    ps = mpsum.tile([P, JT], f32, tag="ps", name="ps")
                    for c in range(KC):
                        for h in range(NH):
                            nc.tensor.matmul(
                                ps[:, h * JW:(h + 1) * JW],
                                xst_sb[:, c, b * P:(b + 1) * P],
                                xt_sb[:, c, jq * JT + h * JW:
                                      jq * JT + (h + 1) * JW],
                                start=(c == 0), stop=(c == KC - 1))
                    e = workp.tile([P, JT], f32, tag="e", name="e")
                    nc.scalar.activation(
                        e, ps[:], Exp, scale=float(inv_T),
                        accum_out=accA[:, b, jq:jq + 1])
                    junk = workp.tile([P, JT], f32, tag="junk", name="junk")
                    nc.vector.scalar_tensor_tensor(
                        out=junk, in0=e, scalar=1.0,
                        in1=m_sb[:, jq * JT:(jq + 1) * JT],
                        op0=mult, op1=mult,
                        accum_out=accM[:, b, jq:jq + 1])
                # tail: logq for block b
                sA = tinyp.tile([P, 1], f32, tag="sA")
                sM = tinyp.tile([P, 1], f32, tag="sM")
                nc.vector.reduce_sum(sA, accA[:, b, :], axis=X)
                nc.vector.reduce_sum(sM, accM[:, b, :], axis=X)
                num = tinyp.tile([P, 1], f32, tag="num")
                den = tinyp.tile([P, 1], f32, tag="den")
                nc.vector.tensor_add(num, sM, cnum_sb[:, b:b + 1])
                nc.vector.tensor_add(den, sA, cden_sb[:, b:b + 1])
                lnn = tinyp.tile([P, 1], f32, tag="lnn")
                lnd = tinyp.tile([P, 1], f32, tag="lnd")
                nc.scalar.activation(lnn, num, Ln)
                nc.scalar.activation(lnd, den, Ln)
                nc.vector.tensor_sub(logq[:, b:b + 1], lnn, lnd)
                nc.sync.dma_start(out_d[b], logq[:, b:b + 1])

    nc.compile()
    return nc


class _Runner:
    """shard_map jit built once; warm calls skip trace/lower/compile."""

    def __init__(self, nc, n_cores):
        import jax
        from jax.sharding import Mesh, PartitionSpec
        try:
            from jax.experimental.shard_map import shard_map
        except ImportError:
            from jax import shard_map
        import concourse.mybir as mybir
        from concourse import bass2jax

        bass2jax.install_neuronx_cc_hook()
        self.n_cores = n_cores
        self.in_names = []
        self.out_names = []
        out_avals = []
        self.zero_outs = []
        partition_name = (nc.partition_id_tensor.name
                          if nc.partition_id_tensor else None)
        for alloc in nc.m.functions[0].allocations:
            if not isinstance(alloc, mybir.MemoryLocationSet):
                continue
            name = alloc.memorylocations[0].name
            if alloc.kind == "ExternalInput":
                if name != partition_name:
                    self.in_names.append(name)
            elif alloc.kind == "ExternalOutput":
                shape = tuple(alloc.tensor_shape)
                dtype = mybir.dt.np(alloc.dtype)
                out_avals.append(jax.core.ShapedArray(shape, dtype))
                self.out_names.append(name)
                self.zero_outs.append(np.zeros(
                    (n_cores * shape[0],) + shape[1:], dtype))
        self.n_params = len(self.in_names)
        all_in = list(self.in_names) + list(self.out_names)
        if partition_name is not None:
            all_in.append(partition_name)
        donate = tuple(range(self.n_params,
                             self.n_params + len(self.out_names)))
        out_avals_t = tuple(out_avals)
        out_names_t = tuple(self.out_names)
        all_in_t = tuple(all_in)
        self.out_shapes = [tuple(a.shape) for a in out_avals]

        def _body(*args):
            operands = list(args)
            if partition_name is not None:
                operands.append(bass2jax.partition_id_tensor())
            outs = bass2jax._bass_exec_p.bind(
                *operands, out_avals=out_avals_t, in_names=all_in_t,
                out_names=out_names_t, lowering_input_output_aliases=(),
                sim_require_finite=True, sim_require_nnan=True, nc=nc)
            return tuple(outs)

        devices = jax.devices()[:n_cores]
        mesh = Mesh(np.asarray(devices), ("core",))
        n_out = len(self.out_names)
        in_specs = (PartitionSpec("core"),) * (self.n_params + n_out)
        out_specs = (PartitionSpec("core"),) * n_out
        self.fn = jax.jit(
            shard_map(_body, mesh=mesh, in_specs=in_specs,
                      out_specs=out_specs, check_rep=False),
            donate_argnums=donate, keep_unused=True)

    def __call__(self, concat_inputs):
        """concat_inputs: name -> global array (n_cores*dim0, ...)."""
        args = [concat_inputs[n] for n in self.in_names]
        zeros = [np.zeros_like(z) for z in self.zero_outs]
        out = self.fn(*args, *zeros)
        return {n: np.asarray(out[i]) for i, n in enumerate(self.out_names)}


def _prepare(inst_embed, anchor, cls_mask, labels, inv_T, n_cores):
    """Host marshalling: normalized bf16 X^T shards, packed masks, folds."""
    N, D = inst_embed.shape
    R = N // n_cores
    NB = R // P
    bf = ml_dtypes.bfloat16
    E0 = float(np.exp(inv_T))

    X = np.ascontiguousarray(inst_embed, dtype=np.float32)
    A = np.ascontiguousarray(anchor, dtype=np.float32)
    nx = np.sqrt(np.einsum("ij,ij->i", X, X))
    na = np.sqrt(np.einsum("ij,ij->i", A, A))
    dot = np.einsum("ij,ij->i", X, A)
    p = np.exp((dot / np.maximum(nx * na, EPS)) * inv_T)

    rinv = 1.0 / np.maximum(nx, 1e-30)
    Xn = (X * rinv[:, None]).astype(bf)                      # [N, D] bf16
    # per-core shards of X-hat^T, stacked: [n_cores*D, R]
    xst_cat = np.ascontiguousarray(
        Xn.reshape(n_cores, R, D).transpose(0, 2, 1)).reshape(n_cores * D, R)

    lab = np.asarray(labels).astype(np.int64)
    C = cls_mask.shape[0]
    # plane-major bit-pack of cls_mask rows: byte k bit b <-> col b*(N/8)+k
    u8 = cls_mask.astype(np.uint8).reshape(C, 8, N // 8)
    pk_cls = np.packbits(u8, axis=1, bitorder="little").reshape(C, N // 8)
    mpk_cat = np.ascontiguousarray(pk_cls[lab])              # [N, N/8]

    mdiag = cls_mask[lab, np.arange(N)].astype(np.float64)
    cnum_cat = (p - E0 * mdiag).astype(np.float32).reshape(
        n_cores * NB, P, 1)
    cden_cat = (p - E0).astype(np.float32).reshape(n_cores * NB, P, 1)
    return {"xst": xst_cat, "mpk": mpk_cat,
            "cnum": cnum_cat, "cden": cden_cat}


def run(inst_embed, anchor, cls_mask, labels, temperature, n_cores=8):
    """Build+compile (cached), run on hardware, reduce. Returns loss f32."""
    from concourse.bass_interp import get_hw_module

    N, D = inst_embed.shape
    R = N // n_cores
    inv_T = float(1.0 / np.float32(temperature))
    key = (N, D, R, inv_T)
    if key not in _CACHE:
        nc = build_kernel(N, D, R, inv_T, n_cores=n_cores)
        nc.m = get_hw_module(nc.m)
        _CACHE[key] = _Runner(nc, n_cores)
    runner = _CACHE[key]

    cat = _prepare(inst_embed, anchor, cls_mask, labels, inv_T, n_cores)
    res = runner(cat)
    vals = np.asarray(res["logq"], dtype=np.float32).reshape(-1)
    loss = -np.mean(vals.astype(np.float64))
    return np.array(loss, dtype=np.float32)


def kernel(inst_embed, anchor, cls_mask, labels, temperature):
    return run(inst_embed, anchor, cls_mask, labels, temperature)


# revision 10
# speedup vs baseline: 35.0899x; 1.0723x over previous
"""Conditional_Embedding_Contrastive_loss Trainium2 kernel (8 cores).

Full-input contract: kernel(**inputs) takes the complete tensors and
returns the scalar loss. End-to-end wall time is dominated by the axon
host->device tunnel (~40-90 MB/s) and per-call jit re-compilation in the
stock runner, so this implementation is built around minimizing both:

  1. Each core receives ONLY its own 1 MB shard of the row-normalized
     embedding matrix (bf16 X-hat^T, [D, R]); the full [D, N] operand is
     assembled on-device with a DRAM AllGather over NeuronLink.
  2. Row norms, the anchor cosine term p_i, and the analytic diagonal
     corrections are computed on the host (cheap O(N*D) numpy) and folded
     into two tiny per-row vectors:
         logq_i = ln(S_msk_i + cnum_i) - ln(S_all_i + cden_i)
     with cnum_i = p_i - exp(1/T)*m_ii, cden_i = p_i - exp(1/T), where
     S_all/S_msk are full-row sums of exp(sim/T) (resp. masked by
     cls_mask[labels_i]) including the diagonal.
  3. The 0/1 mask rows are bit-packed on the host (plane-major: byte k,
     bit b <-> column b*(N/8)+k) to 256 KB/core and unpacked on-device
     with one fused (>>b)&1 DVE op per plane.
  4. The shard_map jit is built once per process and cached, so warm
     calls skip tracing/lowering/compilation entirely.

Device pipeline per core (R = N/8 = 512 rows, P = 128):
  - DRAM AllGather: xst [D,R] -> xg [8*D, R] (logical [8][D][R]).
  - SBUF: xt_sb [128, D/128, N] bf16 (gathered, 8 MB), xst_sb own shard,
    packed mask, cnum/cden.
  - per row-block b (4) and j-tile (1024 cols): PE matmul (8 k-chunks,
    2x512-wide) -> PSUM; ACT exp(scale=1/T) PSUM->SBUF with accum_out =
    unmasked row-sum; DVE scalar_tensor_tensor e*mask with accum_out =
    masked row-sum against the unpacked mask tile.
  - tail per block: two Ln on ACT, subtract, DMA out logq [NB,P,1].
Host: loss = -mean(logq).
"""

import sys

for _p in ("/opt/trn_rl_repo",):
    if _p not in sys.path:
        sys.path.insert(0, _p)

import numpy as np
import ml_dtypes

P = 128          # SBUF partitions
JW = 512         # PE moving free-dim max
EPS = 1e-8

_CACHE = {}


XS = 16.0  # fp8 pre-scale: matmul yields XS^2 * sim, folded out in the exp


def build_kernel(N, D, R, inv_T, n_cores=8, shared_cc_out=True,
                 mpsum_bufs=3, work_bufs=2, mask_bufs=2, x_fp8=True):
    """Build the SPMD Bass program for one core owning R rows of N total."""
    import concourse.bass as bass
    import concourse.mybir as mybir
    import concourse.tile as tile
    from concourse import bacc

    f32 = mybir.dt.float32
    bf16 = mybir.dt.bfloat16
    xdt = mybir.dt.float8e4 if x_fp8 else bf16
    exp_scale = float(inv_T / (XS * XS)) if x_fp8 else float(inv_T)
    u8 = mybir.dt.uint8
    Exp = mybir.ActivationFunctionType.Exp
    Ln = mybir.ActivationFunctionType.Ln
    mult = mybir.AluOpType.mult
    shr = mybir.AluOpType.logical_shift_right
    band = mybir.AluOpType.bitwise_and
    X = mybir.AxisListType.X

    KC = D // P        # contraction chunks of 128
    NB = R // P        # own row blocks
    JT = min(1024, N)  # j-tile width (2 PSUM banks of fp32)
    JC = N // JT       # j tiles per row block
    NH = JT // JW      # matmuls per j-tile per k-chunk
    NPB = N // 8       # packed-mask bytes per row (one bit-plane's width)

    nc = bacc.Bacc(
        "TRN2", target_bir_lowering=False, debug=False, num_devices=n_cores)
    xst_d = nc.declare_dram_parameter("xst", [D, R], xdt, isOutput=False)
    mpk_d = nc.declare_dram_parameter("mpk", [R, NPB], u8, isOutput=False)
    cnum_d = nc.declare_dram_parameter("cnum", [NB, P, 1], f32, isOutput=False)
    cden_d = nc.declare_dram_parameter("cden", [NB, P, 1], f32, isOutput=False)
    out_d = nc.declare_dram_parameter("logq", [NB, P, 1], f32, isOutput=True)

    with tile.TileContext(nc) as tc:
        with (
            tc.tile_pool(name="big", bufs=1) as big,
            tc.tile_pool(name="mask", bufs=mask_bufs) as maskp,
            tc.tile_pool(name="work", bufs=work_bufs) as workp,
            tc.tile_pool(name="stats", bufs=1) as statsp,
            tc.tile_pool(name="tiny", bufs=2) as tinyp,
            tc.tile_pool(name="dram", bufs=1, space="DRAM") as dramp,
            tc.tile_pool(name="mpsum", bufs=mpsum_bufs, space="PSUM") as mpsum,
        ):
            xt_sb = big.tile([P, KC, N], xdt)
            xst_sb = big.tile([P, KC, R], xdt)
            mpk_sb = big.tile([P, NB, NPB], u8)
            cnum_sb = statsp.tile([P, NB], f32)
            cden_sb = statsp.tile([P, NB], f32)
            accA = statsp.tile([P, NB, JC], f32)
            accM = statsp.tile([P, NB, JC], f32)
            logq = statsp.tile([P, NB], f32)

            xin_b = dramp.tile([D, R], xdt)
            xg_b = dramp.tile(
                [n_cores * D, R], xdt,
                addr_space="Shared" if shared_cc_out else "Local")

            # ---- collective: own shard -> full gathered X-hat^T ----
            nc.sync.dma_start(xin_b[:], xst_d[:, :])
            nc.gpsimd.collective_compute(
                "AllGather", mybir.AluOpType.bypass,
                replica_groups=[list(range(n_cores))],
                ins=[xin_b.opt()], outs=[xg_b.opt()])

            # ---- input DMAs that don't depend on the collective ----
            for c in range(KC):
                nc.sync.dma_start(xst_sb[:, c, :], xst_d[c * P:(c + 1) * P, :])
            for b in range(NB):
                nc.sync.dma_start(mpk_sb[:, b, :], mpk_d[b * P:(b + 1) * P, :])
                nc.sync.dma_start(cnum_sb[:, b:b + 1], cnum_d[b])
                nc.sync.dma_start(cden_sb[:, b:b + 1], cden_d[b])

            # Pre-place the combined ln+exp activation table (a table switch
            # costs ~2.7us on the scalar engine).
            ACT_SET_LN_EXP = 6  # natural_log_exp_and_others (gen3 act_info)
            nc.scalar.add_instruction(mybir.InstLoadActFuncSet(
                name=nc.get_next_instruction_name(),
                act_func_set_id=ACT_SET_LN_EXP, ins=[], outs=[]))

            # ---- gathered shard -> SBUF ----
            for k in range(n_cores):
                for c in range(KC):
                    nc.sync.dma_start(
                        xt_sb[:, c, k * R:(k + 1) * R],
                        xg_b[k * D + c * P: k * D + (c + 1) * P, :])

            # ---- main loop ----
            for b in range(NB):
                # unpack this block's mask rows: bit-plane pl covers columns
                # [pl*NPB, (pl+1)*NPB). bitVec TSP ops can't cast dtypes, so
                # (>>pl)&1 stays u8->u8 and a mult-by-1 TSP does u8->bf16.
                m_sb = maskp.tile([P, N], bf16, tag="m", name="m_sb")
                for pl in range(8):
                    msh = maskp.tile([P, NPB], u8, tag="msh", name="msh")
                    nc.vector.tensor_scalar(
                        msh, mpk_sb[:, b, :], pl, 1, op0=shr, op1=band)
                    nc.vector.tensor_scalar_mul(
                        m_sb[:, pl * NPB:(pl + 1) * NPB], msh, 1)
                for jq in range(JC):
                    ps = mpsum.tile([P, JT], f32, tag="ps", name="ps")
                    for c in range(KC):
                        for h in range(NH):
                            nc.tensor.matmul(
                                ps[:, h * JW:(h + 1) * JW],
                                xst_sb[:, c, b * P:(b + 1) * P],
                                xt_sb[:, c, jq * JT + h * JW:
                                      jq * JT + (h + 1) * JW],
                                start=(c == 0), stop=(c == KC - 1))
                    e = workp.tile([P, JT], f32, tag="e", name="e")
                    nc.scalar.activation(
                        e, ps[:], Exp, scale=exp_scale,
                        accum_out=accA[:, b, jq:jq + 1])
                    junk = workp.tile([P, JT], f32, tag="junk", name="junk")
                    nc.vector.scalar_tensor_tensor(
                        out=junk, in0=e, scalar=1.0,
                        in1=m_sb[:, jq * JT:(jq + 1) * JT],
                        op0=mult, op1=mult,
                        accum_out=accM[:, b, jq:jq + 1])
                # tail: logq for block b
                sA = tinyp.tile([P, 1], f32, tag="sA")
                sM = tinyp.tile([P, 1], f32, tag="sM")
                nc.vector.reduce_sum(sA, accA[:, b, :], axis=X)
                nc.vector.reduce_sum(sM, accM[:, b, :], axis=X)
                num = tinyp.tile([P, 1], f32, tag="num")
                den = tinyp.tile([P, 1], f32, tag="den")
                nc.vector.tensor_add(num, sM, cnum_sb[:, b:b + 1])
                nc.vector.tensor_add(den, sA, cden_sb[:, b:b + 1])
                lnn = tinyp.tile([P, 1], f32, tag="lnn")
                lnd = tinyp.tile([P, 1], f32, tag="lnd")
                nc.scalar.activation(lnn, num, Ln)
                nc.scalar.activation(lnd, den, Ln)
                nc.vector.tensor_sub(logq[:, b:b + 1], lnn, lnd)
                nc.sync.dma_start(out_d[b], logq[:, b:b + 1])

    nc.compile()
    return nc


class _Runner:
    """shard_map jit built once; warm calls skip trace/lower/compile."""

    def __init__(self, nc, n_cores):
        import jax
        from jax.sharding import Mesh, PartitionSpec
        try:
            from jax.experimental.shard_map import shard_map
        except ImportError:
            from jax import shard_map
        import concourse.mybir as mybir
        from concourse import bass2jax

        bass2jax.install_neuronx_cc_hook()
        self.n_cores = n_cores
        self.in_names = []
        self.out_names = []
        out_avals = []
        self.zero_outs = []
        partition_name = (nc.partition_id_tensor.name
                          if nc.partition_id_tensor else None)
        for alloc in nc.m.functions[0].allocations:
            if not isinstance(alloc, mybir.MemoryLocationSet):
                continue
            name = alloc.memorylocations[0].name
            if alloc.kind == "ExternalInput":
                if name != partition_name:
                    self.in_names.append(name)
            elif alloc.kind == "ExternalOutput":
                shape = tuple(alloc.tensor_shape)
                dtype = mybir.dt.np(alloc.dtype)
                out_avals.append(jax.core.ShapedArray(shape, dtype))
                self.out_names.append(name)
                self.zero_outs.append(np.zeros(
                    (n_cores * shape[0],) + shape[1:], dtype))
        self.n_params = len(self.in_names)
        all_in = list(self.in_names) + list(self.out_names)
        if partition_name is not None:
            all_in.append(partition_name)
        donate = tuple(range(self.n_params,
                             self.n_params + len(self.out_names)))
        out_avals_t = tuple(out_avals)
        out_names_t = tuple(self.out_names)
        all_in_t = tuple(all_in)
        self.out_shapes = [tuple(a.shape) for a in out_avals]

        def _body(*args):
            operands = list(args)
            if partition_name is not None:
                operands.append(bass2jax.partition_id_tensor())
            outs = bass2jax._bass_exec_p.bind(
                *operands, out_avals=out_avals_t, in_names=all_in_t,
                out_names=out_names_t, lowering_input_output_aliases=(),
                sim_require_finite=True, sim_require_nnan=True, nc=nc)
            return tuple(outs)

        devices = jax.devices()[:n_cores]
        mesh = Mesh(np.asarray(devices), ("core",))
        n_out = len(self.out_names)
        in_specs = (PartitionSpec("core"),) * (self.n_params + n_out)
        out_specs = (PartitionSpec("core"),) * n_out
        from jax.sharding import NamedSharding
        self.sharding = NamedSharding(mesh, PartitionSpec("core"))
        self.fn = jax.jit(
            shard_map(_body, mesh=mesh, in_specs=in_specs,
                      out_specs=out_specs, check_rep=False),
            donate_argnums=donate, keep_unused=True)

    def __call__(self, concat_inputs):
        """concat_inputs: name -> global array (n_cores*dim0, ...)."""
        args = [concat_inputs[n] for n in self.in_names]
        zeros = [np.zeros_like(z) for z in self.zero_outs]
        out = self.fn(*args, *zeros)
        return {n: np.asarray(out[i]) for i, n in enumerate(self.out_names)}


def _prepare(inst_embed, anchor, cls_mask, labels, inv_T, n_cores,
             put=None):
    """Host marshalling: scaled-normalized fp8 X^T shards, packed masks,
    folded correction vectors. If ``put`` is given, each array is handed
    to it as soon as it's ready (async device_put overlaps later prep)."""
    N, D = inst_embed.shape
    R = N // n_cores
    NB = R // P
    E0 = float(np.exp(inv_T))
    if put is None:
        put = lambda a: a
    out = {}

    X = np.ascontiguousarray(inst_embed, dtype=np.float32)
    nx = np.sqrt(np.einsum("ij,ij->i", X, X))
    rinv = XS / np.maximum(nx, 1e-30)
    Xn = (X * rinv[:, None]).astype(ml_dtypes.float8_e4m3)   # [N, D] fp8
    # per-core shards of scaled X-hat^T, stacked: [n_cores*D, R]
    out["xst"] = put(np.ascontiguousarray(
        Xn.reshape(n_cores, R, D).transpose(0, 2, 1)).reshape(n_cores * D, R))

    lab = np.asarray(labels).astype(np.int64)
    C = cls_mask.shape[0]
    # plane-major bit-pack of cls_mask rows: byte k bit b <-> col b*(N/8)+k
    u8 = cls_mask.astype(np.uint8).reshape(C, 8, N // 8)
    pk_cls = np.packbits(u8, axis=1, bitorder="little").reshape(C, N // 8)
    out["mpk"] = put(np.ascontiguousarray(pk_cls[lab]))      # [N, N/8]

    A = np.ascontiguousarray(anchor, dtype=np.float32)
    na = np.sqrt(np.einsum("ij,ij->i", A, A))
    dot = np.einsum("ij,ij->i", X, A)
    p = np.exp((dot / np.maximum(nx * na, EPS)) * inv_T)
    mdiag = cls_mask[lab, np.arange(N)].astype(np.float64)
    out["cnum"] = put((p - E0 * mdiag).astype(np.float32).reshape(
        n_cores * NB, P, 1))
    out["cden"] = put((p - E0).astype(np.float32).reshape(n_cores * NB, P, 1))
    return out


def run(inst_embed, anchor, cls_mask, labels, temperature, n_cores=8):
    """Build+compile (cached), run on hardware, reduce. Returns loss f32."""
    from concourse.bass_interp import get_hw_module

    N, D = inst_embed.shape
    R = N // n_cores
    inv_T = float(1.0 / np.float32(temperature))
    key = (N, D, R, inv_T)
    if key not in _CACHE:
        nc = build_kernel(N, D, R, inv_T, n_cores=n_cores)
        nc.m = get_hw_module(nc.m)
        _CACHE[key] = _Runner(nc, n_cores)
    runner = _CACHE[key]

    import jax
    put = lambda a: jax.device_put(a, runner.sharding)
    cat = _prepare(inst_embed, anchor, cls_mask, labels, inv_T, n_cores,
                   put=put)
    res = runner(cat)
    vals = np.asarray(res["logq"], dtype=np.float32).reshape(-1)
    loss = -np.mean(vals.astype(np.float64))
    return np.array(loss, dtype=np.float32)


def kernel(inst_embed, anchor, cls_mask, labels, temperature):
    return run(inst_embed, anchor, cls_mask, labels, temperature)


# revision 11
# speedup vs baseline: 43.7644x; 1.2472x over previous
"""Conditional_Embedding_Contrastive_loss Trainium2 kernel (8 cores).

Full-input contract: kernel(**inputs) takes the complete tensors and
returns the scalar loss. End-to-end wall time is dominated by the axon
host->device tunnel (~40-90 MB/s) and per-call jit re-compilation in the
stock runner, so this implementation is built around minimizing both:

  1. Each core receives ONLY its own 1 MB shard of the row-normalized
     embedding matrix (bf16 X-hat^T, [D, R]); the full [D, N] operand is
     assembled on-device with a DRAM AllGather over NeuronLink.
  2. Row norms, the anchor cosine term p_i, and the analytic diagonal
     corrections are computed on the host (cheap O(N*D) numpy) and folded
     into two tiny per-row vectors:
         logq_i = ln(S_msk_i + cnum_i) - ln(S_all_i + cden_i)
     with cnum_i = p_i - exp(1/T)*m_ii, cden_i = p_i - exp(1/T), where
     S_all/S_msk are full-row sums of exp(sim/T) (resp. masked by
     cls_mask[labels_i]) including the diagonal.
  3. The 0/1 mask rows are bit-packed on the host (plane-major: byte k,
     bit b <-> column b*(N/8)+k) to 256 KB/core and unpacked on-device
     with one fused (>>b)&1 DVE op per plane.
  4. The shard_map jit is built once per process and cached, so warm
     calls skip tracing/lowering/compilation entirely.

Device pipeline per core (R = N/8 = 512 rows, P = 128):
  - DRAM AllGather: xst [D,R] -> xg [8*D, R] (logical [8][D][R]).
  - SBUF: xt_sb [128, D/128, N] bf16 (gathered, 8 MB), xst_sb own shard,
    packed mask, cnum/cden.
  - per row-block b (4) and j-tile (1024 cols): PE matmul (8 k-chunks,
    2x512-wide) -> PSUM; ACT exp(scale=1/T) PSUM->SBUF with accum_out =
    unmasked row-sum; DVE scalar_tensor_tensor e*mask with accum_out =
    masked row-sum against the unpacked mask tile.
  - tail per block: two Ln on ACT, subtract, DMA out logq [NB,P,1].
Host: loss = -mean(logq).
"""

import sys

for _p in ("/opt/trn_rl_repo",):
    if _p not in sys.path:
        sys.path.insert(0, _p)

import numpy as np
import ml_dtypes

P = 128          # SBUF partitions
JW = 512         # PE moving free-dim max
EPS = 1e-8

_CACHE = {}


XS = 16.0  # fp8 pre-scale: matmul yields XS^2 * sim, folded out in the exp


def build_kernel(N, D, R, inv_T, n_cores=8, shared_cc_out=True,
                 mpsum_bufs=3, work_bufs=2, mask_bufs=2, x_fp8=True):
    """Build the SPMD Bass program for one core owning R rows of N total."""
    import concourse.bass as bass
    import concourse.mybir as mybir
    import concourse.tile as tile
    from concourse import bacc

    f32 = mybir.dt.float32
    bf16 = mybir.dt.bfloat16
    xdt = mybir.dt.float8e4 if x_fp8 else bf16
    exp_scale = float(inv_T / (XS * XS)) if x_fp8 else float(inv_T)
    u8 = mybir.dt.uint8
    Exp = mybir.ActivationFunctionType.Exp
    Ln = mybir.ActivationFunctionType.Ln
    mult = mybir.AluOpType.mult
    shr = mybir.AluOpType.logical_shift_right
    band = mybir.AluOpType.bitwise_and
    X = mybir.AxisListType.X

    KC = D // P        # contraction chunks of 128
    NB = R // P        # own row blocks
    JT = min(1024, N)  # j-tile width (2 PSUM banks of fp32)
    JC = N // JT       # j tiles per row block
    NH = JT // JW      # matmuls per j-tile per k-chunk
    NPB = N // 8       # packed-mask bytes per row (one bit-plane's width)

    nc = bacc.Bacc(
        "TRN2", target_bir_lowering=False, debug=False, num_devices=n_cores)
    xst_d = nc.declare_dram_parameter("xst", [D, R], xdt, isOutput=False)
    mpk_d = nc.declare_dram_parameter("mpk", [R, NPB], u8, isOutput=False)
    cnum_d = nc.declare_dram_parameter("cnum", [NB, P, 1], f32, isOutput=False)
    cden_d = nc.declare_dram_parameter("cden", [NB, P, 1], f32, isOutput=False)
    out_d = nc.declare_dram_parameter("logq", [NB, P, 1], f32, isOutput=True)

    with tile.TileContext(nc) as tc:
        with (
            tc.tile_pool(name="big", bufs=1) as big,
            tc.tile_pool(name="mask", bufs=mask_bufs) as maskp,
            tc.tile_pool(name="work", bufs=work_bufs) as workp,
            tc.tile_pool(name="stats", bufs=1) as statsp,
            tc.tile_pool(name="tiny", bufs=2) as tinyp,
            tc.tile_pool(name="dram", bufs=1, space="DRAM") as dramp,
            tc.tile_pool(name="mpsum", bufs=mpsum_bufs, space="PSUM") as mpsum,
        ):
            xt_sb = big.tile([P, KC, N], xdt)
            xst_sb = big.tile([P, KC, R], xdt)
            mpk_sb = big.tile([P, NB, NPB], u8)
            cnum_sb = statsp.tile([P, NB], f32)
            cden_sb = statsp.tile([P, NB], f32)
            accA = statsp.tile([P, NB, JC], f32)
            accM = statsp.tile([P, NB, JC], f32)
            logq = statsp.tile([P, NB], f32)

            xin_b = dramp.tile([D, R], xdt)
            xg_b = dramp.tile(
                [n_cores * D, R], xdt,
                addr_space="Shared" if shared_cc_out else "Local")

            # ---- collective: own shard -> full gathered X-hat^T ----
            nc.sync.dma_start(xin_b[:], xst_d[:, :])
            nc.gpsimd.collective_compute(
                "AllGather", mybir.AluOpType.bypass,
                replica_groups=[list(range(n_cores))],
                ins=[xin_b.opt()], outs=[xg_b.opt()])

            # ---- input DMAs that don't depend on the collective ----
            for c in range(KC):
                nc.sync.dma_start(xst_sb[:, c, :], xst_d[c * P:(c + 1) * P, :])
            for b in range(NB):
                nc.sync.dma_start(mpk_sb[:, b, :], mpk_d[b * P:(b + 1) * P, :])
                nc.sync.dma_start(cnum_sb[:, b:b + 1], cnum_d[b])
                nc.sync.dma_start(cden_sb[:, b:b + 1], cden_d[b])

            # Pre-place the combined ln+exp activation table (a table switch
            # costs ~2.7us on the scalar engine).
            ACT_SET_LN_EXP = 6  # natural_log_exp_and_others (gen3 act_info)
            nc.scalar.add_instruction(mybir.InstLoadActFuncSet(
                name=nc.get_next_instruction_name(),
                act_func_set_id=ACT_SET_LN_EXP, ins=[], outs=[]))

            # ---- gathered shard -> SBUF ----
            for k in range(n_cores):
                for c in range(KC):
                    nc.sync.dma_start(
                        xt_sb[:, c, k * R:(k + 1) * R],
                        xg_b[k * D + c * P: k * D + (c + 1) * P, :])

            # ---- main loop ----
            for b in range(NB):
                # unpack this block's mask rows: bit-plane pl covers columns
                # [pl*NPB, (pl+1)*NPB). bitVec TSP ops can't cast dtypes, so
                # (>>pl)&1 stays u8->u8 and a mult-by-1 TSP does u8->bf16.
                m_sb = maskp.tile([P, N], bf16, tag="m", name="m_sb")
                for pl in range(8):
                    msh = maskp.tile([P, NPB], u8, tag="msh", name="msh")
                    nc.vector.tensor_scalar(
                        msh, mpk_sb[:, b, :], pl, 1, op0=shr, op1=band)
                    nc.vector.tensor_scalar_mul(
                        m_sb[:, pl * NPB:(pl + 1) * NPB], msh, 1)
                for jq in range(JC):
                    ps = mpsum.tile([P, JT], f32, tag="ps", name="ps")
                    for c in range(KC):
                        for h in range(NH):
                            nc.tensor.matmul(
                                ps[:, h * JW:(h + 1) * JW],
                                xst_sb[:, c, b * P:(b + 1) * P],
                                xt_sb[:, c, jq * JT + h * JW:
                                      jq * JT + (h + 1) * JW],
                                start=(c == 0), stop=(c == KC - 1))
                    e = workp.tile([P, JT], f32, tag="e", name="e")
                    nc.scalar.activation(
                        e, ps[:], Exp, scale=exp_scale,
                        accum_out=accA[:, b, jq:jq + 1])
                    junk = workp.tile([P, JT], f32, tag="junk", name="junk")
                    nc.vector.scalar_tensor_tensor(
                        out=junk, in0=e, scalar=1.0,
                        in1=m_sb[:, jq * JT:(jq + 1) * JT],
                        op0=mult, op1=mult,
                        accum_out=accM[:, b, jq:jq + 1])
                # tail: logq for block b
                sA = tinyp.tile([P, 1], f32, tag="sA")
                sM = tinyp.tile([P, 1], f32, tag="sM")
                nc.vector.reduce_sum(sA, accA[:, b, :], axis=X)
                nc.vector.reduce_sum(sM, accM[:, b, :], axis=X)
                num = tinyp.tile([P, 1], f32, tag="num")
                den = tinyp.tile([P, 1], f32, tag="den")
                nc.vector.tensor_add(num, sM, cnum_sb[:, b:b + 1])
                nc.vector.tensor_add(den, sA, cden_sb[:, b:b + 1])
                lnn = tinyp.tile([P, 1], f32, tag="lnn")
                lnd = tinyp.tile([P, 1], f32, tag="lnd")
                nc.scalar.activation(lnn, num, Ln)
                nc.scalar.activation(lnd, den, Ln)
                nc.vector.tensor_sub(logq[:, b:b + 1], lnn, lnd)
                nc.sync.dma_start(out_d[b], logq[:, b:b + 1])

    nc.compile()
    return nc


class _Runner:
    """shard_map jit built once; warm calls skip trace/lower/compile."""

    def __init__(self, nc, n_cores):
        import jax
        from jax.sharding import Mesh, PartitionSpec
        try:
            from jax.experimental.shard_map import shard_map
        except ImportError:
            from jax import shard_map
        import concourse.mybir as mybir
        from concourse import bass2jax

        bass2jax.install_neuronx_cc_hook()
        self.n_cores = n_cores
        self.in_names = []
        self.out_names = []
        out_avals = []
        self.zero_outs = []
        partition_name = (nc.partition_id_tensor.name
                          if nc.partition_id_tensor else None)
        for alloc in nc.m.functions[0].allocations:
            if not isinstance(alloc, mybir.MemoryLocationSet):
                continue
            name = alloc.memorylocations[0].name
            if alloc.kind == "ExternalInput":
                if name != partition_name:
                    self.in_names.append(name)
            elif alloc.kind == "ExternalOutput":
                shape = tuple(alloc.tensor_shape)
                dtype = mybir.dt.np(alloc.dtype)
                out_avals.append(jax.core.ShapedArray(shape, dtype))
                self.out_names.append(name)
                self.zero_outs.append(np.zeros(
                    (n_cores * shape[0],) + shape[1:], dtype))
        self.n_params = len(self.in_names)
        all_in = list(self.in_names) + list(self.out_names)
        if partition_name is not None:
            all_in.append(partition_name)
        donate = tuple(range(self.n_params,
                             self.n_params + len(self.out_names)))
        out_avals_t = tuple(out_avals)
        out_names_t = tuple(self.out_names)
        all_in_t = tuple(all_in)
        self.out_shapes = [tuple(a.shape) for a in out_avals]

        def _body(*args):
            operands = list(args)
            if partition_name is not None:
                operands.append(bass2jax.partition_id_tensor())
            outs = bass2jax._bass_exec_p.bind(
                *operands, out_avals=out_avals_t, in_names=all_in_t,
                out_names=out_names_t, lowering_input_output_aliases=(),
                sim_require_finite=True, sim_require_nnan=True, nc=nc)
            return tuple(outs)

        devices = jax.devices()[:n_cores]
        mesh = Mesh(np.asarray(devices), ("core",))
        n_out = len(self.out_names)
        in_specs = (PartitionSpec("core"),) * (self.n_params + n_out)
        out_specs = (PartitionSpec("core"),) * n_out
        from jax.sharding import NamedSharding
        self.sharding = NamedSharding(mesh, PartitionSpec("core"))
        self.fn = jax.jit(
            shard_map(_body, mesh=mesh, in_specs=in_specs,
                      out_specs=out_specs, check_rep=False),
            donate_argnums=donate, keep_unused=True)

    def __call__(self, concat_inputs):
        """concat_inputs: name -> global array (n_cores*dim0, ...)."""
        args = [concat_inputs[n] for n in self.in_names]
        zeros = [np.zeros_like(z) for z in self.zero_outs]
        out = self.fn(*args, *zeros)
        return {n: np.asarray(out[i]) for i, n in enumerate(self.out_names)}


_PREP_CACHE = {}


def _get_prep_fns(N, D, C, n_cores, inv_T):
    """Two fused XLA-CPU jits: prep_x (xst shards, put first so its h2d
    overlaps the rest) and prep_rest (packed mask + folded vectors)."""
    key = (N, D, C, n_cores, inv_T)
    if key in _PREP_CACHE:
        return _PREP_CACHE[key]
    import jax
    import jax.numpy as jnp

    R = N // n_cores
    NB = R // P
    E0 = float(np.exp(inv_T))

    def prep_x(X):
        nx2 = jnp.einsum("ij,ij->i", X, X)
        rinv = XS / jnp.maximum(jnp.sqrt(nx2), 1e-30)
        Xn = (X * rinv[:, None]).astype(jnp.float8_e4m3)
        xst = Xn.reshape(n_cores, R, D).transpose(0, 2, 1).reshape(
            n_cores * D, R)
        return xst, nx2

    def prep_rest(X, A, CM, L, nx2):
        # plane-major bit-pack: byte k bit b <-> col b*(N/8)+k
        u8 = CM.astype(jnp.uint8).reshape(C, 8, N // 8)
        pk = (u8 << jnp.arange(8, dtype=jnp.uint8)[None, :, None]).sum(
            1).astype(jnp.uint8)
        mpk = pk[L]
        na2 = jnp.einsum("ij,ij->i", A, A)
        dot = jnp.einsum("ij,ij->i", X, A)
        den = jnp.maximum(jnp.sqrt(nx2) * jnp.sqrt(na2), EPS)
        p = jnp.exp(dot / den * inv_T)
        md = CM[L, jnp.arange(N)].astype(jnp.float32)
        cnum = (p - E0 * md).astype(jnp.float32).reshape(n_cores * NB, P, 1)
        cden = (p - E0).astype(jnp.float32).reshape(n_cores * NB, P, 1)
        return mpk, cnum, cden

    fns = (jax.jit(prep_x), jax.jit(prep_rest))
    _PREP_CACHE[key] = fns
    return fns


def _prepare(inst_embed, anchor, cls_mask, labels, inv_T, n_cores,
             put=None):
    """Host marshalling. If ``put`` is given, each array is handed to it
    as soon as it's ready (async device_put overlaps later prep)."""
    import jax

    N, D = inst_embed.shape
    C = cls_mask.shape[0]
    if put is None:
        put = lambda a: np.asarray(a)
    prep_x, prep_rest = _get_prep_fns(N, D, C, n_cores, inv_T)

    X = np.ascontiguousarray(inst_embed, dtype=np.float32)
    A = np.ascontiguousarray(anchor, dtype=np.float32)
    L = np.asarray(labels)
    CM = np.ascontiguousarray(cls_mask, dtype=np.int32)
    cpu = jax.devices("cpu")[0]
    out = {}
    with jax.default_device(cpu):
        xst, nx2 = prep_x(X)
        out["xst"] = put(xst)
        mpk, cnum, cden = prep_rest(X, A, CM, L, nx2)
    out["mpk"] = put(mpk)
    out["cnum"] = put(cnum)
    out["cden"] = put(cden)
    return out


def run(inst_embed, anchor, cls_mask, labels, temperature, n_cores=8):
    """Build+compile (cached), run on hardware, reduce. Returns loss f32."""
    from concourse.bass_interp import get_hw_module

    N, D = inst_embed.shape
    R = N // n_cores
    inv_T = float(1.0 / np.float32(temperature))
    key = (N, D, R, inv_T)
    if key not in _CACHE:
        nc = build_kernel(N, D, R, inv_T, n_cores=n_cores)
        nc.m = get_hw_module(nc.m)
        _CACHE[key] = _Runner(nc, n_cores)
    runner = _CACHE[key]

    import jax
    put = lambda a: jax.device_put(a, runner.sharding)
    cat = _prepare(inst_embed, anchor, cls_mask, labels, inv_T, n_cores,
                   put=put)
    res = runner(cat)
    vals = np.asarray(res["logq"], dtype=np.float32).reshape(-1)
    loss = -np.mean(vals.astype(np.float64))
    return np.array(loss, dtype=np.float32)


def kernel(inst_embed, anchor, cls_mask, labels, temperature):
    return run(inst_embed, anchor, cls_mask, labels, temperature)


# revision 12
# speedup vs baseline: 50.7416x; 1.1594x over previous
"""Conditional_Embedding_Contrastive_loss Trainium2 kernel (8 cores).

Full-input contract: kernel(**inputs) takes the complete tensors and
returns the scalar loss. End-to-end wall time is dominated by the axon
host->device tunnel (~40-90 MB/s) and per-call jit re-compilation in the
stock runner, so this implementation is built around minimizing both:

  1. Each core receives ONLY its own 1 MB shard of the row-normalized
     embedding matrix (bf16 X-hat^T, [D, R]); the full [D, N] operand is
     assembled on-device with a DRAM AllGather over NeuronLink.
  2. Row norms, the anchor cosine term p_i, and the analytic diagonal
     corrections are computed on the host (cheap O(N*D) numpy) and folded
     into two tiny per-row vectors:
         logq_i = ln(S_msk_i + cnum_i) - ln(S_all_i + cden_i)
     with cnum_i = p_i - exp(1/T)*m_ii, cden_i = p_i - exp(1/T), where
     S_all/S_msk are full-row sums of exp(sim/T) (resp. masked by
     cls_mask[labels_i]) including the diagonal.
  3. The 0/1 mask rows are bit-packed on the host (plane-major: byte k,
     bit b <-> column b*(N/8)+k) to 256 KB/core and unpacked on-device
     with one fused (>>b)&1 DVE op per plane.
  4. The shard_map jit is built once per process and cached, so warm
     calls skip tracing/lowering/compilation entirely.

Device pipeline per core (R = N/8 = 512 rows, P = 128):
  - DRAM AllGather: xst [D,R] -> xg [8*D, R] (logical [8][D][R]).
  - SBUF: xt_sb [128, D/128, N] bf16 (gathered, 8 MB), xst_sb own shard,
    packed mask, cnum/cden.
  - per row-block b (4) and j-tile (1024 cols): PE matmul (8 k-chunks,
    2x512-wide) -> PSUM; ACT exp(scale=1/T) PSUM->SBUF with accum_out =
    unmasked row-sum; DVE scalar_tensor_tensor e*mask with accum_out =
    masked row-sum against the unpacked mask tile.
  - tail per block: two Ln on ACT, subtract, DMA out logq [NB,P,1].
Host: loss = -mean(logq).
"""

import sys

for _p in ("/opt/trn_rl_repo",):
    if _p not in sys.path:
        sys.path.insert(0, _p)

import numpy as np
import ml_dtypes

P = 128          # SBUF partitions
JW = 512         # PE moving free-dim max
EPS = 1e-8

_CACHE = {}


XS = 16.0  # fp8 pre-scale: matmul yields XS^2 * sim, folded out in the exp


def build_kernel(N, D, R, inv_T, n_cores=8, shared_cc_out=True,
                 mpsum_bufs=3, work_bufs=2, mask_bufs=2, x_fp8=True):
    """Build the SPMD Bass program for one core owning R rows of N total."""
    import concourse.bass as bass
    import concourse.mybir as mybir
    import concourse.tile as tile
    from concourse import bacc

    f32 = mybir.dt.float32
    bf16 = mybir.dt.bfloat16
    xdt = mybir.dt.float8e4 if x_fp8 else bf16
    exp_scale = float(inv_T / (XS * XS)) if x_fp8 else float(inv_T)
    u8 = mybir.dt.uint8
    Exp = mybir.ActivationFunctionType.Exp
    Ln = mybir.ActivationFunctionType.Ln
    mult = mybir.AluOpType.mult
    shr = mybir.AluOpType.logical_shift_right
    band = mybir.AluOpType.bitwise_and
    X = mybir.AxisListType.X

    KC = D // P        # contraction chunks of 128
    NB = R // P        # own row blocks
    JT = min(1024, N)  # j-tile width (2 PSUM banks of fp32)
    JC = N // JT       # j tiles per row block
    NH = JT // JW      # matmuls per j-tile per k-chunk
    NPB = N // 8       # packed-mask bytes per row (one bit-plane's width)

    nc = bacc.Bacc(
        "TRN2", target_bir_lowering=False, debug=False, num_devices=n_cores)
    xst_d = nc.declare_dram_parameter("xst", [D, R], xdt, isOutput=False)
    mpk_d = nc.declare_dram_parameter("mpk", [R, NPB], u8, isOutput=False)
    cnum_d = nc.declare_dram_parameter("cnum", [NB, P, 1], f32, isOutput=False)
    cden_d = nc.declare_dram_parameter("cden", [NB, P, 1], f32, isOutput=False)
    out_d = nc.declare_dram_parameter("logq", [NB, P, 1], f32, isOutput=True)

    with tile.TileContext(nc) as tc:
        with (
            tc.tile_pool(name="big", bufs=1) as big,
            tc.tile_pool(name="mask", bufs=mask_bufs) as maskp,
            tc.tile_pool(name="work", bufs=work_bufs) as workp,
            tc.tile_pool(name="stats", bufs=1) as statsp,
            tc.tile_pool(name="tiny", bufs=2) as tinyp,
            tc.tile_pool(name="dram", bufs=1, space="DRAM") as dramp,
            tc.tile_pool(name="mpsum", bufs=mpsum_bufs, space="PSUM") as mpsum,
        ):
            xt_sb = big.tile([P, KC, N], xdt)
            xst_sb = big.tile([P, KC, R], xdt)
            mpk_sb = big.tile([P, NB, NPB], u8)
            cnum_sb = statsp.tile([P, NB], f32)
            cden_sb = statsp.tile([P, NB], f32)
            accA = statsp.tile([P, NB, JC], f32)
            accM = statsp.tile([P, NB, JC], f32)
            logq = statsp.tile([P, NB], f32)

            xin_b = dramp.tile([D, R], xdt)
            xg_b = dramp.tile(
                [n_cores * D, R], xdt,
                addr_space="Shared" if shared_cc_out else "Local")

            # ---- collective: own shard -> full gathered X-hat^T ----
            nc.sync.dma_start(xin_b[:], xst_d[:, :])
            nc.gpsimd.collective_compute(
                "AllGather", mybir.AluOpType.bypass,
                replica_groups=[list(range(n_cores))],
                ins=[xin_b.opt()], outs=[xg_b.opt()])

            # ---- input DMAs that don't depend on the collective ----
            for c in range(KC):
                nc.sync.dma_start(xst_sb[:, c, :], xst_d[c * P:(c + 1) * P, :])
            for b in range(NB):
                nc.sync.dma_start(mpk_sb[:, b, :], mpk_d[b * P:(b + 1) * P, :])
                nc.sync.dma_start(cnum_sb[:, b:b + 1], cnum_d[b])
                nc.sync.dma_start(cden_sb[:, b:b + 1], cden_d[b])

            # Pre-place the combined ln+exp activation table (a table switch
            # costs ~2.7us on the scalar engine).
            ACT_SET_LN_EXP = 6  # natural_log_exp_and_others (gen3 act_info)
            nc.scalar.add_instruction(mybir.InstLoadActFuncSet(
                name=nc.get_next_instruction_name(),
                act_func_set_id=ACT_SET_LN_EXP, ins=[], outs=[]))

            # ---- gathered shard -> SBUF ----
            for k in range(n_cores):
                for c in range(KC):
                    nc.sync.dma_start(
                        xt_sb[:, c, k * R:(k + 1) * R],
                        xg_b[k * D + c * P: k * D + (c + 1) * P, :])

            # ---- main loop ----
            for b in range(NB):
                # unpack this block's mask rows: bit-plane pl covers columns
                # [pl*NPB, (pl+1)*NPB). bitVec TSP ops can't cast dtypes, so
                # (>>pl)&1 stays u8->u8 and a mult-by-1 TSP does u8->bf16.
                m_sb = maskp.tile([P, N], bf16, tag="m", name="m_sb")
                for pl in range(8):
                    msh = maskp.tile([P, NPB], u8, tag="msh", name="msh")
                    nc.vector.tensor_scalar(
                        msh, mpk_sb[:, b, :], pl, 1, op0=shr, op1=band)
                    nc.vector.tensor_scalar_mul(
                        m_sb[:, pl * NPB:(pl + 1) * NPB], msh, 1)
                for jq in range(JC):
                    ps = mpsum.tile([P, JT], f32, tag="ps", name="ps")
                    for c in range(KC):
                        for h in range(NH):
                            nc.tensor.matmul(
                                ps[:, h * JW:(h + 1) * JW],
                                xst_sb[:, c, b * P:(b + 1) * P],
                                xt_sb[:, c, jq * JT + h * JW:
                                      jq * JT + (h + 1) * JW],
                                start=(c == 0), stop=(c == KC - 1))
                    e = workp.tile([P, JT], f32, tag="e", name="e")
                    nc.scalar.activation(
                        e, ps[:], Exp, scale=exp_scale,
                        accum_out=accA[:, b, jq:jq + 1])
                    junk = workp.tile([P, JT], f32, tag="junk", name="junk")
                    nc.vector.scalar_tensor_tensor(
                        out=junk, in0=e, scalar=1.0,
                        in1=m_sb[:, jq * JT:(jq + 1) * JT],
                        op0=mult, op1=mult,
                        accum_out=accM[:, b, jq:jq + 1])
                # tail: logq for block b
                sA = tinyp.tile([P, 1], f32, tag="sA")
                sM = tinyp.tile([P, 1], f32, tag="sM")
                nc.vector.reduce_sum(sA, accA[:, b, :], axis=X)
                nc.vector.reduce_sum(sM, accM[:, b, :], axis=X)
                num = tinyp.tile([P, 1], f32, tag="num")
                den = tinyp.tile([P, 1], f32, tag="den")
                nc.vector.tensor_add(num, sM, cnum_sb[:, b:b + 1])
                nc.vector.tensor_add(den, sA, cden_sb[:, b:b + 1])
                lnn = tinyp.tile([P, 1], f32, tag="lnn")
                lnd = tinyp.tile([P, 1], f32, tag="lnd")
                nc.scalar.activation(lnn, num, Ln)
                nc.scalar.activation(lnd, den, Ln)
                nc.vector.tensor_sub(logq[:, b:b + 1], lnn, lnd)
                nc.sync.dma_start(out_d[b], logq[:, b:b + 1])

    nc.compile()
    return nc


class _Runner:
    """shard_map jit built once; warm calls skip trace/lower/compile."""

    def __init__(self, nc, n_cores):
        import jax
        from jax.sharding import Mesh, PartitionSpec
        try:
            from jax.experimental.shard_map import shard_map
        except ImportError:
            from jax import shard_map
        import concourse.mybir as mybir
        from concourse import bass2jax

        bass2jax.install_neuronx_cc_hook()
        self.n_cores = n_cores
        self.in_names = []
        self.out_names = []
        out_avals = []
        self.zero_outs = []
        partition_name = (nc.partition_id_tensor.name
                          if nc.partition_id_tensor else None)
        for alloc in nc.m.functions[0].allocations:
            if not isinstance(alloc, mybir.MemoryLocationSet):
                continue
            name = alloc.memorylocations[0].name
            if alloc.kind == "ExternalInput":
                if name != partition_name:
                    self.in_names.append(name)
            elif alloc.kind == "ExternalOutput":
                shape = tuple(alloc.tensor_shape)
                dtype = mybir.dt.np(alloc.dtype)
                out_avals.append(jax.core.ShapedArray(shape, dtype))
                self.out_names.append(name)
                self.zero_outs.append(np.zeros(
                    (n_cores * shape[0],) + shape[1:], dtype))
        self.n_params = len(self.in_names)
        all_in = list(self.in_names) + list(self.out_names)
        if partition_name is not None:
            all_in.append(partition_name)
        donate = tuple(range(self.n_params,
                             self.n_params + len(self.out_names)))
        out_avals_t = tuple(out_avals)
        out_names_t = tuple(self.out_names)
        all_in_t = tuple(all_in)
        self.out_shapes = [tuple(a.shape) for a in out_avals]

        def _body(*args):
            operands = list(args)
            if partition_name is not None:
                operands.append(bass2jax.partition_id_tensor())
            outs = bass2jax._bass_exec_p.bind(
                *operands, out_avals=out_avals_t, in_names=all_in_t,
                out_names=out_names_t, lowering_input_output_aliases=(),
                sim_require_finite=True, sim_require_nnan=True, nc=nc)
            return tuple(outs)

        devices = jax.devices()[:n_cores]
        mesh = Mesh(np.asarray(devices), ("core",))
        n_out = len(self.out_names)
        in_specs = (PartitionSpec("core"),) * (self.n_params + n_out)
        out_specs = (PartitionSpec("core"),) * n_out
        from jax.sharding import NamedSharding
        self.sharding = NamedSharding(mesh, PartitionSpec("core"))
        self.fn = jax.jit(
            shard_map(_body, mesh=mesh, in_specs=in_specs,
                      out_specs=out_specs, check_rep=False),
            donate_argnums=donate, keep_unused=True)

    def __call__(self, concat_inputs):
        """concat_inputs: name -> global array (n_cores*dim0, ...)."""
        args = [concat_inputs[n] for n in self.in_names]
        zeros = [np.zeros_like(z) for z in self.zero_outs]
        out = self.fn(*args, *zeros)
        return {n: np.asarray(out[i]) for i, n in enumerate(self.out_names)}


_PREP_CACHE = {}


def _get_prep_fns(N, D, C, n_cores, inv_T):
    """Two fused XLA-CPU jits: prep_x (xst shards, put first so its h2d
    overlaps the rest) and prep_rest (packed mask + folded vectors)."""
    key = (N, D, C, n_cores, inv_T)
    if key in _PREP_CACHE:
        return _PREP_CACHE[key]
    import jax
    import jax.numpy as jnp

    R = N // n_cores
    NB = R // P
    E0 = float(np.exp(inv_T))

    def prep_x(X):
        nx2 = jnp.einsum("ij,ij->i", X, X)
        rinv = XS / jnp.maximum(jnp.sqrt(nx2), 1e-30)
        Xn = (X * rinv[:, None]).astype(jnp.float8_e4m3)
        xst = Xn.reshape(n_cores, R, D).transpose(0, 2, 1).reshape(
            n_cores * D, R)
        return xst, nx2

    def prep_rest(X, A, CM, L, nx2):
        # plane-major bit-pack: byte k bit b <-> col b*(N/8)+k
        u8 = CM.astype(jnp.uint8).reshape(C, 8, N // 8)
        pk = (u8 << jnp.arange(8, dtype=jnp.uint8)[None, :, None]).sum(
            1).astype(jnp.uint8)
        mpk = pk[L]
        na2 = jnp.einsum("ij,ij->i", A, A)
        dot = jnp.einsum("ij,ij->i", X, A)
        den = jnp.maximum(jnp.sqrt(nx2) * jnp.sqrt(na2), EPS)
        p = jnp.exp(dot / den * inv_T)
        md = CM[L, jnp.arange(N)].astype(jnp.float32)
        cnum = (p - E0 * md).astype(jnp.float32).reshape(n_cores * NB, P, 1)
        cden = (p - E0).astype(jnp.float32).reshape(n_cores * NB, P, 1)
        return mpk, cnum, cden

    fns = (jax.jit(prep_x), jax.jit(prep_rest))
    _PREP_CACHE[key] = fns
    return fns


def _prepare(inst_embed, anchor, cls_mask, labels, inv_T, n_cores,
             put=None):
    """Host marshalling. If ``put`` is given, each array is handed to it
    as soon as it's ready (async device_put overlaps later prep)."""
    import jax

    N, D = inst_embed.shape
    C = cls_mask.shape[0]
    if put is None:
        put = lambda a: np.asarray(a)
    prep_x, prep_rest = _get_prep_fns(N, D, C, n_cores, inv_T)

    X = np.ascontiguousarray(inst_embed, dtype=np.float32)
    A = np.ascontiguousarray(anchor, dtype=np.float32)
    L = np.asarray(labels)
    CM = np.ascontiguousarray(cls_mask, dtype=np.int32)
    cpu = jax.devices("cpu")[0]
    out = {}
    with jax.default_device(cpu):
        xst, nx2 = prep_x(X)
        # device_put of a LAZY cpu array blocks on its compute; materialize
        # first so the put dispatches async and the h2d overlaps prep_rest.
        xst.block_until_ready()
        out["xst"] = put(xst)
        mpk, cnum, cden = prep_rest(X, A, CM, L, nx2)
        mpk.block_until_ready()
    out["mpk"] = put(mpk)
    out["cnum"] = put(cnum)
    out["cden"] = put(cden)
    return out


def run(inst_embed, anchor, cls_mask, labels, temperature, n_cores=8):
    """Build+compile (cached), run on hardware, reduce. Returns loss f32."""
    from concourse.bass_interp import get_hw_module

    N, D = inst_embed.shape
    R = N // n_cores
    inv_T = float(1.0 / np.float32(temperature))
    key = (N, D, R, inv_T)
    if key not in _CACHE:
        nc = build_kernel(N, D, R, inv_T, n_cores=n_cores)
        nc.m = get_hw_module(nc.m)
        _CACHE[key] = _Runner(nc, n_cores)
    runner = _CACHE[key]

    import jax
    put = lambda a: jax.device_put(a, runner.sharding)
    cat = _prepare(inst_embed, anchor, cls_mask, labels, inv_T, n_cores,
                   put=put)
    res = runner(cat)
    vals = np.asarray(res["logq"], dtype=np.float32).reshape(-1)
    loss = -np.mean(vals.astype(np.float64))
    return np.array(loss, dtype=np.float32)


def kernel(inst_embed, anchor, cls_mask, labels, temperature):
    return run(inst_embed, anchor, cls_mask, labels, temperature)
